# revision 17
# baseline (speedup 1.0000x reference)
"""TRN2 Bass kernel for nn_LSTMModelTrig: LSTM(1->50, T=2048) + FC(50->1).

Contract: kernel(**inputs) takes the FULL inputs from setup_inputs() and
returns the FULL [8192, 1] output, sharding batch across 8 NeuronCores
internally (data-parallel; weights replicated; no cross-core comms).

Per-core architecture (B_local = 1024 = G groups x J tiles x 128):
  - batch on partitions; gates/features on the free dim.
  - h_sb [128, J, 64] bf16: cols 0:50 h, 50 x_t, 51 ones, 52:64 zeros.
  - step: xcol copy (gpsimd) -> DVE 32x32 block-transpose -> block-diagonal
    32x32 bf16 matmuls (tile_position=(32i,32i), 2 K-chunks accumulate in
    PSUM) -> ONE sigmoid over all 200 gate cols on ScalarE (gate order
    [i,f,o,g], g pre-acts scaled 2x in the weights: tanh(x)=2*sigmoid(2x)-1)
    -> cell update with c~ = c/2 (exact): m1 = (sig_g-0.5)*i via custom DVE
    op, m2 = f*c~ on GpSimd, c~ = m1+m2 on DVE (bf16 2x mode) ->
    h = o*tanh(2c~) via a custom DVE op evaluating a deg-5 odd minimax
    polynomial (fit on |2c~|<=1.3; actual |2c~| < ~0.6) fused with the
    o-gate multiply -> next step.
  - dummy matmuls (scratch PSUM) keep the PE HAM clock at 8/8.
  - final: out = sum_k h[:,k]*W_fc[k] via scalar_tensor_tensor accum;
    b_fc added on host.
"""

import sys

sys.path.insert(0, "/opt/trn_rl_repo")

import numpy as np

import concourse.bacc as bacc
import concourse.bass as bass
import concourse.mybir as mybir
import concourse.tile as tile
from concourse.bass_utils import run_bass_kernel_spmd

FP32 = mybir.dt.float32
BF16 = mybir.dt.bfloat16
AF = mybir.ActivationFunctionType
ALU = mybir.AluOpType

H = 50
GATES = 200
NPAD = 256
T_FULL = 2048
B_FULL = 8192
N_CORES = 8
import os as _os
J = int(_os.environ.get("LSTM_J", "4"))
G = int(_os.environ.get("LSTM_G", "2"))
U = int(_os.environ.get("LSTM_U", "256"))
M2_GPSIMD = _os.environ.get("LSTM_M2_GPSIMD", "1") == "1"
TANH_CUSTOM = _os.environ.get("LSTM_TANH_CUSTOM", "1") == "1"
M1_CUSTOM = _os.environ.get("LSTM_M1_CUSTOM", "1") == "1"
N_DUMMY = int(_os.environ.get("LSTM_DUMMY", "6"))
DUMMY_N = int(_os.environ.get("LSTM_DUMMY_N", "64"))
CELL_SPLIT = _os.environ.get("LSTM_CELL_SPLIT", "0") == "1"

# deg-5 odd minimax fit of tanh(y) on y in [-1.3, 1.3] (y = 2*c~), factored
# as T(x) = (u + C1)*u*w + v with v = x*S, w = x*TT, u = v^2  (x = c~)
TANH2MUL_S = 1.984142440
TANH2MUL_C1 = -5.136287446
TANH2MUL_T = 0.113085003

# ---- custom DVE ops (registered into concourse.dve_ops at import) -------- #


def _register_custom_ops():
    from concourse.dve_spec import Spec, Src0, Src1, C0, C1, C2, sq, lower
    from concourse.dve_spec import _has_src1
    import concourse.dve_ops as dops
    from concourse.dve_uop import DveOpSpec

    def register(name, spec):
        if name in dops._SUB_OPCODE_FOR_NAME:
            return next(o for o in dops.OPS if o.name == name)
        row = dops._CUSTOM_DVE_ROW_BASE + len(dops.OPS)
        assert row < 0x20
        dops._SUB_OPCODE_FOR_NAME[name] = row
        shas = {}
        for ver in ("v3", "v4"):
            s = DveOpSpec(name=name, opcode=row, uops=lower(spec, ver=ver),
                          rd1_en=_has_src1(spec))
            shas[ver] = s.sha(ver)
        op = dops.DveOp(name, spec, subdim=False, uops_sha=shas)
        dops.OPS.append(op)
        dops.CUSTOM_DVE_SPECS[name] = spec
        return op

    # h = Src0 * T(Src1): Src0 = o-gate, Src1 = c~ (in1 must be <=1 free dim)
    _v = Src1 * C0
    _w = Src1 * C2
    _u = sq(_v)
    tanh2mul = register(
        "LSTM_TANH2MUL",
        Spec(
            body=(((_u + C1) * _u) * _w + _v) * Src0,
            reference=lambda in0, in1, s0, s1, imm2: (
                lambda v, w, u: (((u + s1) * u) * w + v)
                * in0.reshape(v.shape).astype(np.float32)
            )(
                in1.reshape(in0.shape).astype(np.float32) * s0,
                in1.reshape(in0.shape).astype(np.float32) * imm2,
                (in1.reshape(in0.shape).astype(np.float32) * s0) ** 2,
            ),
        ),
    )

    # m1 = (Src0 - 0.5) * Src1: Src0 = sig_g, Src1 = i
    m1sub = register(
        "LSTM_M1SUB",
        Spec(
            body=(Src0 - C0) * Src1,
            reference=lambda in0, in1, s0, s1, imm2: (
                (in0.astype(np.float32) - s0)
                * in1.reshape(in0.shape).astype(np.float32)
            ),
        ),
    )
    return tanh2mul, m1sub


TANH2MUL_OP, M1SUB_OP = _register_custom_ops()

_nc_cache = {}


def _build_nc(T=T_FULL):
    key = (T, J, G, U, M2_GPSIMD, TANH_CUSTOM, M1_CUSTOM, N_DUMMY, DUMMY_N,
           CELL_SPLIT)
    if key in _nc_cache:
        return _nc_cache[key]
    nc = bacc.Bacc("TRN2", target_bir_lowering=False, debug=False)
    B_local = 128 * J * G
    x_dram = nc.dram_tensor("x", [B_local, T], FP32, kind="ExternalInput")
    wr0_dram = nc.dram_tensor("wr0", [128, GATES], FP32, kind="ExternalInput")
    wr1_dram = nc.dram_tensor("wr1", [128, GATES], FP32, kind="ExternalInput")
    wfc_dram = nc.dram_tensor("wfcb", [128, H], FP32, kind="ExternalInput")
    out_dram = nc.dram_tensor("out", [128, J * G], FP32, kind="ExternalOutput")

    with tile.TileContext(nc) as tc:
        with (
            tc.tile_pool(name="const", bufs=1) as constp,
            tc.tile_pool(name="state", bufs=1) as statep,
            tc.tile_pool(name="xbuf", bufs=2) as xp,
            tc.tile_pool(name="psum", bufs=1, space="PSUM") as psp,
        ):
            wr_f = [constp.tile([128, GATES], FP32, tag="wr0f", name="wr0f"),
                    constp.tile([128, GATES], FP32, tag="wr1f", name="wr1f")]
            nc.sync.dma_start(wr_f[0][:], wr0_dram[:])
            nc.sync.dma_start(wr_f[1][:], wr1_dram[:])
            wfcb = constp.tile([128, H], FP32, tag="wfcb", name="wfcb")
            nc.sync.dma_start(wfcb[:], wfc_dram[:])

            wr_hi = [constp.tile([128, GATES], BF16, tag="wrh0", name="wrh0"),
                     constp.tile([128, GATES], BF16, tag="wrh1", name="wrh1")]
            for kb in range(2):
                nc.vector.tensor_copy(wr_hi[kb][:], wr_f[kb][:])

            h_sb, bt, c_sb, sg, tc_sb, m1, m2, ps = ([] for _ in range(8))
            for g in range(G):
                h_sb.append(statep.tile([128, J, 64], BF16, tag=f"h{g}", name=f"h{g}"))
                bt.append(statep.tile([128, J, 64], BF16, tag=f"bt{g}", name=f"bt{g}"))
                c_sb.append(statep.tile([128, J * H], BF16, tag=f"c{g}", name=f"c{g}"))
                sg.append(statep.tile([128, J, GATES], BF16, tag=f"s{g}", name=f"s{g}"))
                tc_sb.append(statep.tile([128, J, H], BF16, tag=f"tc{g}", name=f"tc{g}"))
                m1.append(statep.tile([128, J * H], BF16, tag=f"m1{g}", name=f"m1{g}"))
                m2.append(statep.tile([128, J * H], BF16, tag=f"m2{g}", name=f"m2{g}"))
                # two PSUM tiles per group (one per j-pair) so the sigmoid
                # of the first pair can start while the second pair's
                # matmuls still stream (hazards are tracked per tile)
                ps.append([psp.tile([128, J // 2, NPAD], FP32,
                                    tag=f"ps{g}h{h}", name=f"ps{g}h{h}")
                           for h in range(2)])
                nc.vector.memset(h_sb[g][:], 0.0)
                nc.vector.memset(c_sb[g][:], 0.0)
                nc.vector.memset(h_sb[g][:, :, 51:52], 1.0)
            ps_dm = psp.tile([128, 128], FP32, tag="psdm", name="psdm")

            def step_body(g, xs, u):
                hg, btg, cg, sgg = h_sb[g], bt[g], c_sb[g], sg[g]
                nc.gpsimd.tensor_copy(hg[:, :, 50:51], xs[:, :, u : u + 1])
                nc.vector.transpose(btg[:], hg[:])
                jh = J // 2
                for j in range(J):
                    for kb in range(2):
                        for i in range(4):
                            p0 = 32 * i
                            nc.tensor.matmul(
                                ps[g][j // jh][p0 : p0 + 32, j % jh, 0:GATES],
                                btg[p0 : p0 + 32, j, 32 * kb : 32 * kb + 32],
                                wr_hi[kb][p0 : p0 + 32, :],
                                start=(kb == 0),
                                stop=(kb == 1),
                                tile_position=(p0, p0),
                            )
                for _ in range(N_DUMMY):
                    nc.tensor.matmul(
                        ps_dm[0:32, 0:DUMMY_N],
                        wr_hi[0][0:32, 0:32],
                        wr_hi[0][0:32, 0:DUMMY_N],
                        start=True, stop=True,
                        tile_position=(0, 0),
                    )
                # gate order [i, f, o, g]; g pre-acts carry 2x from weights.
                # One sigmoid per j-pair: the first overlaps the second
                # pair's matmul waves (separate PSUM tiles).
                for h in range(2):
                    nc.scalar.activation(
                        sgg[:, h * jh : (h + 1) * jh, :],
                        ps[g][h][:, :, 0:GATES], AF.Sigmoid)
                # m2 = f * c~  split per j-pair so the first half starts
                # right after sigma_h0 (it is the add's binding input)
                for h in range(2):
                    (nc.gpsimd if M2_GPSIMD else nc.vector).tensor_mul(
                        m2[g][:, h * jh * H : (h + 1) * jh * H],
                        sgg[:, h * jh : (h + 1) * jh, 50:100],
                        cg[:, h * jh * H : (h + 1) * jh * H])
                # m1 = (sig_g - 0.5) * i; c~ = m1 + m2; h = o * tanh(2*c~)
                halves = ([(h * jh, (h + 1) * jh) for h in range(2)]
                          if CELL_SPLIT else [(0, J)])
                for (a, b) in halves:
                    sl = slice(a * H, b * H)
                    if M1_CUSTOM:
                        nc.vector._custom_dve(
                            M1SUB_OP, out=m1[g][:, sl],
                            in0=sgg[:, a:b, 150:200], in1=sgg[:, a:b, 0:50],
                            s0=0.5)
                    else:
                        nc.vector.scalar_tensor_tensor(
                            m1[g][:, sl], sgg[:, a:b, 150:200], 0.5,
                            sgg[:, a:b, 0:50], ALU.subtract, ALU.mult)
                    # c~ = m1 + m2  (bf16 dense: DVE 2x mode)
                    nc.vector.tensor_add(cg[:, sl], m1[g][:, sl], m2[g][:, sl])
                    if TANH_CUSTOM:
                        nc.vector._custom_dve(
                            TANH2MUL_OP, out=hg[:, a:b, 0:50],
                            in0=sgg[:, a:b, 100:150], in1=cg[:, sl],
                            s0=TANH2MUL_S, s1=TANH2MUL_C1, imm2=TANH2MUL_T)
                    else:
                        nc.scalar.activation(
                            tc_sb[g][:, a:b, :], cg[:, sl], AF.Tanh, scale=2.0)
                        nc.vector.tensor_mul(
                            hg[:, a:b, 0:50], sgg[:, a:b, 100:150],
                            tc_sb[g][:, a:b, :])

            def iteration(iv):
                xs_list = []
                for g in range(G):
                    xs = xp.tile([128, J, U], FP32, tag=f"x{g}", name=f"xs{g}")
                    for j in range(J):
                        jt = g * J + j
                        nc.sync.dma_start(
                            xs[:, j, :],
                            x_dram[128 * jt : 128 * (jt + 1), bass.ds(iv, U)],
                        )
                    xs_list.append(xs)
                for u in range(U):
                    for g in range(G):
                        step_body(g, xs_list[g], u)

            if T // U == 1:
                iteration(0)
            else:
                with tc.For_i(0, T, U, hint_engines=tuple(mybir.ALL_ENGINES)) as iv:
                    iteration(iv)

            out_sb = statep.tile([128, J * G], FP32, tag="out", name="out_sb")
            scratch = statep.tile([128, H], FP32, tag="scratch", name="scratch")
            for g in range(G):
                for j in range(J):
                    jt = g * J + j
                    nc.vector.scalar_tensor_tensor(
                        scratch[:],
                        h_sb[g][:, j, 0:50],
                        0.0,
                        wfcb[:],
                        ALU.add,
                        ALU.mult,
                        accum_out=out_sb[:, jt : jt + 1],
                    )
            nc.sync.dma_start(out_dram[:], out_sb[:])

    nc.compile()
    _nc_cache[key] = nc
    return nc


def _make_weights(W_ih, W_hh, b_ih, b_hh, W_fc):
    # torch gate order [i, f, g, o] -> ours [i, f, o, g]
    perm = np.concatenate([np.arange(0, 100), np.arange(150, 200),
                           np.arange(100, 150)])
    # g block (cols 150:200 after perm) scaled 2x: tanh(x) = 2*sigmoid(2x)-1
    gscale = np.ones(GATES, np.float32)
    gscale[150:200] = 2.0
    w_aug = np.zeros((64, GATES), np.float32)
    w_aug[0:50, :] = W_hh.T[:, perm] * gscale
    w_aug[50, :] = W_ih[perm, 0] * gscale
    w_aug[51, :] = (b_ih + b_hh)[perm] * gscale
    wr0 = np.tile(w_aug[0:32], (4, 1)).astype(np.float32)
    wr1 = np.tile(w_aug[32:64], (4, 1)).astype(np.float32)
    wfcb = np.tile(W_fc[0:1, :].astype(np.float32), (128, 1))
    return wr0, wr1, wfcb


def _run(nc, x_shards, wr0, wr1, wfcb, trace=False, **kw):
    in_maps = [
        {"x": xs, "wr0": wr0, "wr1": wr1, "wfcb": wfcb} for xs in x_shards
    ]
    return run_bass_kernel_spmd(nc, in_maps, list(range(len(x_shards))),
                                trace=trace, **kw)


def kernel(x, W_ih, W_hh, b_ih, b_hh, W_fc, b_fc, _trace=False, **_kw):
    x = np.ascontiguousarray(np.asarray(x, dtype=np.float32).reshape(B_FULL, T_FULL))
    wr0, wr1, wfcb = _make_weights(
        np.asarray(W_ih, np.float32), np.asarray(W_hh, np.float32),
        np.asarray(b_ih, np.float32), np.asarray(b_hh, np.float32),
        np.asarray(W_fc, np.float32))
    nc = _build_nc()
    B_local = B_FULL // N_CORES
    x_shards = [np.ascontiguousarray(x[c * B_local:(c + 1) * B_local])
                for c in range(N_CORES)]
    res = _run(nc, x_shards, wr0, wr1, wfcb, trace=_trace, **_kw)
    outs = []
    for c in range(N_CORES):
        outs.append(res.results[c]["out"].T.reshape(-1))  # b_local = 128*jt + p
    out = np.concatenate(outs) + np.float32(b_fc[0])
    if _trace:
        kernel.last_results = res
    return out.reshape(B_FULL, 1).astype(np.float32)


# revision 24
# speedup vs baseline: 1.1096x; 1.1096x over previous
"""TRN2 Bass kernel for nn_LSTMModelTrig: LSTM(1->50, T=2048) + FC(50->1).

Contract: kernel(**inputs) takes the FULL inputs from setup_inputs() and
returns the FULL [8192, 1] output, sharding batch across 8 NeuronCores
internally (data-parallel; weights replicated; no cross-core comms).

Per-core architecture (B_local = 1024 = G groups x J tiles x 128):
  - batch on partitions; gates/features on the free dim.
  - h_sb [128, J, 64] bf16: cols 0:50 h, 50 x_t, 51 ones, 52:64 zeros.
  - step: xcol copy (gpsimd) -> DVE 32x32 block-transpose -> block-diagonal
    32x32 bf16 matmuls (tile_position=(32i,32i), 2 K-chunks accumulate in
    PSUM) -> ONE sigmoid over all 200 gate cols on ScalarE (gate order
    [i,f,o,g], g pre-acts scaled 2x in the weights: tanh(x)=2*sigmoid(2x)-1)
    -> cell update with c~ = c/2 (exact): m1 = (sig_g-0.5)*i via custom DVE
    op, m2 = f*c~ on GpSimd, c~ = m1+m2 on DVE (bf16 2x mode) ->
    h = o*tanh(2c~) via a custom DVE op evaluating a deg-5 odd minimax
    polynomial (fit on |2c~|<=1.3; actual |2c~| < ~0.6) fused with the
    o-gate multiply -> next step.
  - dummy matmuls (scratch PSUM) keep the PE HAM clock at 8/8.
  - final: out = sum_k h[:,k]*W_fc[k] via scalar_tensor_tensor accum;
    b_fc added on host.
"""

import sys

sys.path.insert(0, "/opt/trn_rl_repo")

import numpy as np

import concourse.bacc as bacc
import concourse.bass as bass
import concourse.mybir as mybir
import concourse.tile as tile
from concourse.bass_utils import run_bass_kernel_spmd

FP32 = mybir.dt.float32
BF16 = mybir.dt.bfloat16
AF = mybir.ActivationFunctionType
ALU = mybir.AluOpType

H = 50
GATES = 200
NPAD = 256
T_FULL = 2048
B_FULL = 8192
N_CORES = 8
import os as _os
J = int(_os.environ.get("LSTM_J", "4"))
G = int(_os.environ.get("LSTM_G", "2"))
U = int(_os.environ.get("LSTM_U", "256"))
M2_GPSIMD = _os.environ.get("LSTM_M2_GPSIMD", "1") == "1"
TANH_CUSTOM = _os.environ.get("LSTM_TANH_CUSTOM", "1") == "1"
M1_CUSTOM = _os.environ.get("LSTM_M1_CUSTOM", "1") == "1"
N_DUMMY = int(_os.environ.get("LSTM_DUMMY", "6"))
DUMMY_N = int(_os.environ.get("LSTM_DUMMY_N", "64"))
CELL_SPLIT = _os.environ.get("LSTM_CELL_SPLIT", "0") == "1"
PSPLIT = int(_os.environ.get("LSTM_PSPLIT", "2"))  # psum tiles per group

# deg-5 odd minimax fit of tanh(y) on y in [-1.3, 1.3] (y = 2*c~), factored
# as T(x) = (u + C1)*u*w + v with v = x*S, w = x*TT, u = v^2  (x = c~)
TANH2MUL_S = 1.984142440
TANH2MUL_C1 = -5.136287446
TANH2MUL_T = 0.113085003

# ---- custom DVE ops (registered into concourse.dve_ops at import) -------- #


def _register_custom_ops():
    from concourse.dve_spec import Spec, Src0, Src1, C0, C1, C2, sq, lower
    from concourse.dve_spec import _has_src1
    import concourse.dve_ops as dops
    from concourse.dve_uop import DveOpSpec

    def register(name, spec):
        if name in dops._SUB_OPCODE_FOR_NAME:
            return next(o for o in dops.OPS if o.name == name)
        row = dops._CUSTOM_DVE_ROW_BASE + len(dops.OPS)
        assert row < 0x20
        dops._SUB_OPCODE_FOR_NAME[name] = row
        shas = {}
        for ver in ("v3", "v4"):
            s = DveOpSpec(name=name, opcode=row, uops=lower(spec, ver=ver),
                          rd1_en=_has_src1(spec))
            shas[ver] = s.sha(ver)
        op = dops.DveOp(name, spec, subdim=False, uops_sha=shas)
        dops.OPS.append(op)
        dops.CUSTOM_DVE_SPECS[name] = spec
        return op

    # h = Src0 * T(Src1): Src0 = o-gate, Src1 = c~ (in1 must be <=1 free dim)
    _v = Src1 * C0
    _w = Src1 * C2
    _u = sq(_v)
    tanh2mul = register(
        "LSTM_TANH2MUL",
        Spec(
            body=(((_u + C1) * _u) * _w + _v) * Src0,
            reference=lambda in0, in1, s0, s1, imm2: (
                lambda v, w, u: (((u + s1) * u) * w + v)
                * in0.reshape(v.shape).astype(np.float32)
            )(
                in1.reshape(in0.shape).astype(np.float32) * s0,
                in1.reshape(in0.shape).astype(np.float32) * imm2,
                (in1.reshape(in0.shape).astype(np.float32) * s0) ** 2,
            ),
        ),
    )

    # m1 = (Src0 - 0.5) * Src1: Src0 = sig_g, Src1 = i
    m1sub = register(
        "LSTM_M1SUB",
        Spec(
            body=(Src0 - C0) * Src1,
            reference=lambda in0, in1, s0, s1, imm2: (
                (in0.astype(np.float32) - s0)
                * in1.reshape(in0.shape).astype(np.float32)
            ),
        ),
    )
    return tanh2mul, m1sub


TANH2MUL_OP, M1SUB_OP = _register_custom_ops()

_nc_cache = {}


def _build_nc(T=T_FULL):
    key = (T, J, G, U, M2_GPSIMD, TANH_CUSTOM, M1_CUSTOM, N_DUMMY, DUMMY_N,
           CELL_SPLIT, PSPLIT)
    if key in _nc_cache:
        return _nc_cache[key]
    nc = bacc.Bacc("TRN2", target_bir_lowering=False, debug=False)
    B_local = 128 * J * G
    x_dram = nc.dram_tensor("x", [B_local, T], FP32, kind="ExternalInput")
    wr0_dram = nc.dram_tensor("wr0", [128, GATES], FP32, kind="ExternalInput")
    wr1_dram = nc.dram_tensor("wr1", [128, GATES], FP32, kind="ExternalInput")
    wfc_dram = nc.dram_tensor("wfcb", [128, H], FP32, kind="ExternalInput")
    out_dram = nc.dram_tensor("out", [128, J * G], FP32, kind="ExternalOutput")

    with tile.TileContext(nc) as tc:
        with (
            tc.tile_pool(name="const", bufs=1) as constp,
            tc.tile_pool(name="state", bufs=1) as statep,
            tc.tile_pool(name="xbuf", bufs=2) as xp,
            tc.tile_pool(name="psum", bufs=1, space="PSUM") as psp,
        ):
            wr_f = [constp.tile([128, GATES], FP32, tag="wr0f", name="wr0f"),
                    constp.tile([128, GATES], FP32, tag="wr1f", name="wr1f")]
            nc.sync.dma_start(wr_f[0][:], wr0_dram[:])
            nc.sync.dma_start(wr_f[1][:], wr1_dram[:])
            wfcb = constp.tile([128, H], FP32, tag="wfcb", name="wfcb")
            nc.sync.dma_start(wfcb[:], wfc_dram[:])

            wr_hi = [constp.tile([128, GATES], BF16, tag="wrh0", name="wrh0"),
                     constp.tile([128, GATES], BF16, tag="wrh1", name="wrh1")]
            for kb in range(2):
                nc.vector.tensor_copy(wr_hi[kb][:], wr_f[kb][:])

            h_sb, bt, c_sb, sg, tc_sb, m1, m2, ps = ([] for _ in range(8))
            for g in range(G):
                h_sb.append(statep.tile([128, J, 64], BF16, tag=f"h{g}", name=f"h{g}"))
                bt.append(statep.tile([128, J, 64], BF16, tag=f"bt{g}", name=f"bt{g}"))
                c_sb.append(statep.tile([128, J * H], BF16, tag=f"c{g}", name=f"c{g}"))
                sg.append(statep.tile([128, J, GATES], BF16, tag=f"s{g}", name=f"s{g}"))
                tc_sb.append(statep.tile([128, J, H], BF16, tag=f"tc{g}", name=f"tc{g}"))
                m1.append(statep.tile([128, J * H], BF16, tag=f"m1{g}", name=f"m1{g}"))
                m2.append(statep.tile([128, J * H], BF16, tag=f"m2{g}", name=f"m2{g}"))
                # PSPLIT PSUM tiles per group (one per j-chunk) so earlier
                # chunks' sigmoids start while later chunks' matmuls still
                # stream (hazards are tracked per tile)
                ps.append([psp.tile([128, J // PSPLIT, NPAD], FP32,
                                    tag=f"ps{g}h{h}", name=f"ps{g}h{h}")
                           for h in range(PSPLIT)])
                nc.vector.memset(h_sb[g][:], 0.0)
                nc.vector.memset(c_sb[g][:], 0.0)
                nc.vector.memset(h_sb[g][:, :, 51:52], 1.0)
            # separate dummy psum tile when bank budget allows; else aim
            # dummies at the unused pad cols of a real tile (range-disjoint
            # from the 0:200 gate reads, so no false hazards)
            ps_dm = (psp.tile([128, 128], FP32, tag="psdm", name="psdm")
                     if PSPLIT * G + 1 <= 8 else None)

            def step_body(g, xs, u):
                hg, btg, cg, sgg = h_sb[g], bt[g], c_sb[g], sg[g]
                nc.gpsimd.tensor_copy(hg[:, :, 50:51], xs[:, :, u : u + 1])
                nc.vector.transpose(btg[:], hg[:])
                jh = J // PSPLIT
                for j in range(J):
                    for kb in range(2):
                        for i in range(4):
                            p0 = 32 * i
                            nc.tensor.matmul(
                                ps[g][j // jh][p0 : p0 + 32, j % jh, 0:GATES],
                                btg[p0 : p0 + 32, j, 32 * kb : 32 * kb + 32],
                                wr_hi[kb][p0 : p0 + 32, :],
                                start=(kb == 0),
                                stop=(kb == 1),
                                tile_position=(p0, p0),
                            )
                dn = DUMMY_N if ps_dm is not None else min(DUMMY_N, NPAD - GATES - 8)
                dtgt = (ps_dm[0:32, 0:dn] if ps_dm is not None
                        else ps[g][0][0:32, 0, GATES : GATES + dn])
                for _ in range(N_DUMMY):
                    nc.tensor.matmul(
                        dtgt,
                        wr_hi[0][0:32, 0:32],
                        wr_hi[0][0:32, 0:dn],
                        start=True, stop=True,
                        tile_position=(0, 0),
                    )
                # gate order [i, f, o, g]; g pre-acts carry 2x from weights.
                # One sigmoid per j-chunk: earlier chunks overlap later
                # chunks' matmul waves (separate PSUM tiles).
                for h in range(PSPLIT):
                    nc.scalar.activation(
                        sgg[:, h * jh : (h + 1) * jh, :],
                        ps[g][h][:, :, 0:GATES], AF.Sigmoid)
                # m2 = f * c~  split per j-chunk so early halves start
                # right after their sigmoid (it is the add's binding input)
                for h in range(PSPLIT):
                    (nc.gpsimd if M2_GPSIMD else nc.vector).tensor_mul(
                        m2[g][:, h * jh * H : (h + 1) * jh * H],
                        sgg[:, h * jh : (h + 1) * jh, 50:100],
                        cg[:, h * jh * H : (h + 1) * jh * H])
                # m1 = (sig_g - 0.5) * i; c~ = m1 + m2; h = o * tanh(2*c~)
                halves = ([(h * jh, (h + 1) * jh) for h in range(2)]
                          if CELL_SPLIT else [(0, J)])
                for (a, b) in halves:
                    sl = slice(a * H, b * H)
                    if M1_CUSTOM:
                        nc.vector._custom_dve(
                            M1SUB_OP, out=m1[g][:, sl],
                            in0=sgg[:, a:b, 150:200], in1=sgg[:, a:b, 0:50],
                            s0=0.5)
                    else:
                        nc.vector.scalar_tensor_tensor(
                            m1[g][:, sl], sgg[:, a:b, 150:200], 0.5,
                            sgg[:, a:b, 0:50], ALU.subtract, ALU.mult)
                    # c~ = m1 + m2  (bf16 dense: DVE 2x mode)
                    nc.vector.tensor_add(cg[:, sl], m1[g][:, sl], m2[g][:, sl])
                    if TANH_CUSTOM:
                        nc.vector._custom_dve(
                            TANH2MUL_OP, out=hg[:, a:b, 0:50],
                            in0=sgg[:, a:b, 100:150], in1=cg[:, sl],
                            s0=TANH2MUL_S, s1=TANH2MUL_C1, imm2=TANH2MUL_T)
                    else:
                        nc.scalar.activation(
                            tc_sb[g][:, a:b, :], cg[:, sl], AF.Tanh, scale=2.0)
                        nc.vector.tensor_mul(
                            hg[:, a:b, 0:50], sgg[:, a:b, 100:150],
                            tc_sb[g][:, a:b, :])

            def iteration(iv):
                xs_list = []
                for g in range(G):
                    xs = xp.tile([128, J, U], FP32, tag=f"x{g}", name=f"xs{g}")
                    for j in range(J):
                        jt = g * J + j
                        nc.sync.dma_start(
                            xs[:, j, :],
                            x_dram[128 * jt : 128 * (jt + 1), bass.ds(iv, U)],
                        )
                    xs_list.append(xs)
                for u in range(U):
                    for g in range(G):
                        step_body(g, xs_list[g], u)

            if T // U == 1:
                iteration(0)
            else:
                with tc.For_i(0, T, U, hint_engines=tuple(mybir.ALL_ENGINES)) as iv:
                    iteration(iv)

            out_sb = statep.tile([128, J * G], FP32, tag="out", name="out_sb")
            scratch = statep.tile([128, H], FP32, tag="scratch", name="scratch")
            for g in range(G):
                for j in range(J):
                    jt = g * J + j
                    nc.vector.scalar_tensor_tensor(
                        scratch[:],
                        h_sb[g][:, j, 0:50],
                        0.0,
                        wfcb[:],
                        ALU.add,
                        ALU.mult,
                        accum_out=out_sb[:, jt : jt + 1],
                    )
            nc.sync.dma_start(out_dram[:], out_sb[:])

    nc.compile()
    _nc_cache[key] = nc
    return nc


def _make_weights(W_ih, W_hh, b_ih, b_hh, W_fc):
    # torch gate order [i, f, g, o] -> ours [i, f, o, g]
    perm = np.concatenate([np.arange(0, 100), np.arange(150, 200),
                           np.arange(100, 150)])
    # g block (cols 150:200 after perm) scaled 2x: tanh(x) = 2*sigmoid(2x)-1
    gscale = np.ones(GATES, np.float32)
    gscale[150:200] = 2.0
    w_aug = np.zeros((64, GATES), np.float32)
    w_aug[0:50, :] = W_hh.T[:, perm] * gscale
    w_aug[50, :] = W_ih[perm, 0] * gscale
    w_aug[51, :] = (b_ih + b_hh)[perm] * gscale
    wr0 = np.tile(w_aug[0:32], (4, 1)).astype(np.float32)
    wr1 = np.tile(w_aug[32:64], (4, 1)).astype(np.float32)
    wfcb = np.tile(W_fc[0:1, :].astype(np.float32), (128, 1))
    return wr0, wr1, wfcb


def _run(nc, x_shards, wr0, wr1, wfcb, trace=False, **kw):
    in_maps = [
        {"x": xs, "wr0": wr0, "wr1": wr1, "wfcb": wfcb} for xs in x_shards
    ]
    return run_bass_kernel_spmd(nc, in_maps, list(range(len(x_shards))),
                                trace=trace, **kw)


def kernel(x, W_ih, W_hh, b_ih, b_hh, W_fc, b_fc, _trace=False, **_kw):
    x = np.ascontiguousarray(np.asarray(x, dtype=np.float32).reshape(B_FULL, T_FULL))
    wr0, wr1, wfcb = _make_weights(
        np.asarray(W_ih, np.float32), np.asarray(W_hh, np.float32),
        np.asarray(b_ih, np.float32), np.asarray(b_hh, np.float32),
        np.asarray(W_fc, np.float32))
    nc = _build_nc()
    B_local = B_FULL // N_CORES
    x_shards = [np.ascontiguousarray(x[c * B_local:(c + 1) * B_local])
                for c in range(N_CORES)]
    res = _run(nc, x_shards, wr0, wr1, wfcb, trace=_trace, **_kw)
    outs = []
    for c in range(N_CORES):
        outs.append(res.results[c]["out"].T.reshape(-1))  # b_local = 128*jt + p
    out = np.concatenate(outs) + np.float32(b_fc[0])
    if _trace:
        kernel.last_results = res
    return out.reshape(B_FULL, 1).astype(np.float32)
